# revision 1
# baseline (speedup 1.0000x reference)
"""MultiHeadAttention Trainium2 kernel.

B=2, S=2048, E=1024, H=16, D=64. 8 NeuronCores.

Sharding: B*H = 32 (batch, head) pairs -> 4 heads per core (core c handles
batch c//4, heads 4*(c%4)..4*(c%4)+3). Out-projection is column-sharded by
head (Wo folded with Wv); partial [S, E] outputs are summed on host (the
"all-reduce"), each core adding bo/4 so the sum carries the bias exactly once.

Math (per head h):
  S_scores = (q @ Wq.T) @ (k @ Wk.T).T / sqrt(D)  ==  q @ (A/8) @ k.T,
    A = Wq.T @ Wk  (so q needs no projection on device)
  P = softmax(mask(S_scores))  (unnormalized exp + ones-column trick)
  ctx = P @ v  (raw v; Wv folded into Wo)
  out_h = ctx @ (Wo[:, cols_h] @ Wv).T

Device layout: scores computed transposed, S.T[sk, sq] tiles, so that
exp(S.T) feeds the ctx matmul directly as the moving operand and the
ones-column of v_aug produces the softmax denominators r[sq] as row 64 of
the ctx accumulator. Normalization multiplies by broadcast 1/r before the
output projection.
"""

import sys

if "/opt/trn_rl_repo" not in sys.path:
    sys.path.insert(0, "/opt/trn_rl_repo")

import numpy as np

import concourse.bass as bass
import concourse.tile as tile
from concourse import bacc, mybir
from concourse.bass_utils import run_bass_kernel_spmd

B, S, E, H = 2, 2048, 1024, 16
D = E // H  # 64
N_CORES = 8
HEADS_PER_CORE = H * B // N_CORES  # 4
N_CHUNK = 4  # sq chunks of 512
CHUNK = S // N_CHUNK  # 512
N_BLK = S // 128  # 16 sk blocks of 128
F32 = mybir.dt.float32
F32R = mybir.dt.float32r


def _analyze_mask(mask):
    """Classify each (sq-chunk, sk-block) region of the shared mask.

    Returns (schedule, tiles): schedule[ci] is a list of (blk, mode, aux)
    with mode in {"plain", "causal", "tile"}; tiles is the list of distinct
    float32 [128, CHUNK] (sk, sq) multiplicative mask tiles for "tile" mode.
    """
    m = np.asarray(mask).reshape(S, S) != 0
    schedule = []
    tiles = []
    tile_index = {}
    for ci in range(N_CHUNK):
        q0 = ci * CHUNK
        blks = []
        for k in range(N_BLK):
            k0 = k * 128
            mb = m[q0 : q0 + CHUNK, k0 : k0 + 128]  # [sq, sk]
            if not mb.any():
                continue
            if mb.all():
                blks.append((k, "plain", None))
                continue
            causal = (
                np.arange(q0, q0 + CHUNK)[:, None] >= np.arange(k0, k0 + 128)[None, :]
            )
            if np.array_equal(mb, causal):
                blks.append((k, "causal", None))
            else:
                t = np.ascontiguousarray(mb.T.astype(np.float32))  # [sk, sq]
                key = t.tobytes()
                if key not in tile_index:
                    tile_index[key] = len(tiles)
                    tiles.append(t)
                blks.append((k, "tile", tile_index[key]))
        schedule.append(blks)
    return schedule, tiles


import os

SP_BUFS = int(os.environ.get("K_SP_BUFS", "3"))
PP_BUFS = int(os.environ.get("K_PP_BUFS", "2"))
ES_BUFS = int(os.environ.get("K_ES_BUFS", "10"))
OUTP_DELAY = int(os.environ.get("K_OUTP_DELAY", "0"))


def build_nc(schedule, n_mask_tiles, repeat=1, hw_loop=0):
    """Build the SPMD Bass program (identical for all 8 cores).

    repeat>1 re-executes the whole data path (input DMAs included) that many
    times in one NEFF; used by test.py to measure per-execution device time
    as a wall-clock slope.
    """
    nc = bacc.Bacc(
        "TRN2", target_bir_lowering=False, debug=False, num_devices=N_CORES
    )

    qT_d = nc.dram_tensor("qT", [2, 128, S], F32, kind="ExternalInput").ap()
    kT_d = nc.dram_tensor("kT", [2, 128, S], F32, kind="ExternalInput").ap()
    va_d = nc.dram_tensor("va", [4, 128, N_BLK * (D + 1)], F32, kind="ExternalInput").ap()
    wq_d = nc.dram_tensor("wq", [D, D], F32, kind="ExternalInput").ap()
    wk_d = nc.dram_tensor("wk", [D, D], F32, kind="ExternalInput").ap()
    wv_d = nc.dram_tensor("wv", [D, D], F32, kind="ExternalInput").ap()
    woT_d = nc.dram_tensor("woT", [4, D, E], F32, kind="ExternalInput").ap()
    bo4_d = nc.dram_tensor("bo4", [1, E], F32, kind="ExternalInput").ap()
    if n_mask_tiles:
        mt_d = nc.dram_tensor(
            "mtiles", [n_mask_tiles, 128, CHUNK], F32, kind="ExternalInput"
        ).ap()
    out_d = nc.dram_tensor("out", [S, E], F32, kind="ExternalOutput").ap()

    Exp = mybir.ActivationFunctionType.Exp

    from contextlib import ExitStack

    with tile.TileContext(nc) as tc, ExitStack() as ctx:
        const = ctx.enter_context(tc.tile_pool(name="const", bufs=1))
        qk = ctx.enter_context(tc.tile_pool(name="qk", bufs=1))
        va_pool = ctx.enter_context(tc.tile_pool(name="vap", bufs=1))
        es_pool = ctx.enter_context(tc.tile_pool(name="es", bufs=ES_BUFS))
        nrm = ctx.enter_context(tc.tile_pool(name="nrm", bufs=3))
        outp = ctx.enter_context(tc.tile_pool(name="outp", bufs=3))
        sp = ctx.enter_context(tc.tile_pool(name="sp", bufs=SP_BUFS, space="PSUM"))
        pp = ctx.enter_context(tc.tile_pool(name="pp", bufs=PP_BUFS, space="PSUM"))

        # ---- constants / weight prep ----
        wq_sb = const.tile([D, D], F32, tag="wq")
        # Wk loaded twice side by side: the A.T matmul then yields A.T
        # replicated on partitions 0-63 and 64-127 in one shot (matches
        # either head of a pair-stacked rhs, no SBUF->SBUF copy needed)
        wk2_sb = const.tile([D, 2 * D], F32, tag="wk2")
        wv_sb = const.tile([D, D], F32R, tag="wv")
        nc.sync.dma_start(wq_sb[:], wq_d[:])
        nc.sync.dma_start(wk2_sb[:, 0:D], wk_d[:])
        nc.sync.dma_start(wk2_sb[:, D : 2 * D], wk_d[:])
        nc.sync.dma_start(wv_sb[:], wv_d[:].bitcast(F32R))

        # A.T/8 = (Wk.T @ Wq)/8  [d', d], replicated over both partition halves
        at_ps = pp.tile([128, D], F32, tag="ctx")
        nc.tensor.matmul(at_ps[:], wk2_sb[:], wq_sb[:], start=True, stop=True)
        at_sb = const.tile([128, D], F32R, tag="at")
        nc.vector.tensor_scalar_mul(at_sb[:], at_ps[:], 1.0 / np.sqrt(float(D)))

        wovT, mtiles = [], []
        bo4_bc = None

        def _emit_prep():
            nonlocal bo4_bc
            # ---- deferred weight prep (not needed until first outP) ----
            for p in range(2):
                wovT_p = const.tile(
                    [128, E], F32R, tag=f"wovT{p}", name=f"wovT{p}"
                )
                wovT.append(wovT_p)
            for h in range(4):
                woT_sb = const.tile([D, E], F32R, tag="woT_ld")
                nc.scalar.dma_start(woT_sb[:], woT_d[h].bitcast(F32R))
                p, o = h // 2, (h % 2) * D
                for ec in range(E // 512):
                    wo_ps = pp.tile([D, 512], F32, tag="ctx")
                    nc.tensor.matmul(
                        wo_ps[:],
                        wv_sb[:],
                        woT_sb[:, ec * 512 : (ec + 1) * 512],
                        start=True,
                        stop=True,
                    )
                    nc.vector.tensor_copy(
                        wovT[p][o : o + D, ec * 512 : (ec + 1) * 512], wo_ps[:]
                    )
            bo4_sb = const.tile([1, E], F32, tag="bo4")
            nc.scalar.dma_start(bo4_sb[:], bo4_d[:])
            bo4_bc = const.tile([128, E], F32, tag="bo4bc")
            nc.gpsimd.partition_broadcast(bo4_bc[:], bo4_sb[:])
            for i in range(n_mask_tiles):
                t = const.tile([128, CHUNK], F32R, tag=f"mt{i}", name=f"mt{i}")
                nc.scalar.dma_start(t[:], mt_d[i].bitcast(F32R))
                mtiles.append(t)


        def _emit_body(_first):
            # ---- input loads, ci-major; kA is emitted per-chunk in the
            # main loop so PE never waits on later chunks' DMA ----
            qT = []
            kAT = []
            va = []
            k_sb_l = []
            for p in range(2):
                qT.append(qk.tile([128, S], F32R, tag=f"qT{p}", name=f"qT{p}"))
                k_sb_l.append(qk.tile([128, S], F32R, tag=f"kT{p}", name=f"kT{p}"))
                kAT.append(qk.tile([128, S], F32R, tag=f"kAT{p}", name=f"kAT{p}"))
            for h in range(4):
                v_sb = va_pool.tile(
                    [128, N_BLK * (D + 1)], F32R, tag=f"va{h}", name=f"va{h}"
                )
                va.append(v_sb)
            for ci in range(N_CHUNK):
                cs = slice(ci * CHUNK, (ci + 1) * CHUNK)
                for p in range(2):
                    nc.scalar.dma_start(
                        k_sb_l[p][:, cs], kT_d[p, :, cs].bitcast(F32R)
                    )
                    nc.sync.dma_start(qT[p][:, cs], qT_d[p, :, cs].bitcast(F32R))
                if ci < 2:
                    for hh in range(2):
                        h = 2 * ci + hh
                        nc.gpsimd.dma_start(va[h][:], va_d[h].bitcast(F32R))

            if _first and not hw_loop:
                _emit_prep()

            # ---- main loop ----
            prev_out = None  # (ctxN pair tiles, q0) of the previous chunk

            def emit_outp(ctxN_prev, q0_prev):
                for sb in range(CHUNK // 128):
                    ls = slice(sb * 128, (sb + 1) * 128)
                    for ec in range(E // 512):
                        es_ = slice(ec * 512, (ec + 1) * 512)
                        o_ps = pp.tile([128, 512], F32, tag="ctx", name="o_ps")
                        nc.tensor.matmul(
                            o_ps[:],
                            ctxN_prev[0][:, ls],
                            wovT[0][:, es_],
                            start=True,
                            stop=False,
                        )
                        nc.tensor.matmul(
                            o_ps[:],
                            ctxN_prev[1][:, ls],
                            wovT[1][:, es_],
                            start=False,
                            stop=True,
                        )
                        o_sb = outp.tile([128, 512], F32, tag="osb", name="o_sb")
                        nc.vector.tensor_add(o_sb[:], o_ps[:], bo4_bc[:, es_])
                        _odma = (
                            nc.scalar if os.environ.get("K_OUT_RING") == "act" else nc.sync
                        )
                        _odma.dma_start(
                            out_d[q0_prev + sb * 128 : q0_prev + (sb + 1) * 128, es_],
                            o_sb[:],
                        )

            for ci in range(N_CHUNK):
                blks = schedule[ci]
                q0 = ci * CHUNK
                # per-block compute-column start: causal diagonal blocks only
                # have valid entries at cols >= k0-q0; keep matmul N >= 256
                # for the fp32r fast path.
                def c0_of(blk, mode):
                    if mode != "causal":
                        return 0, 0
                    c0 = max(0, blk * 128 - q0)  # first possibly-valid col
                    return min(c0, CHUNK - 256), c0

                # just-in-time kA.T for this chunk's new sk columns
                cs_k = slice(q0, q0 + CHUNK)
                for p_ in range(2):
                    for hh in range(2):
                        o = hh * D
                        ka_ps = sp.tile([D, CHUNK], F32, tag="scores", name="ka_ps")
                        nc.tensor.matmul(
                            ka_ps[:],
                            at_sb[o : o + D, :],
                            k_sb_l[p_][o : o + D, cs_k],
                            start=True,
                            stop=True,
                        )
                        nc.vector.tensor_copy(kAT[p_][o : o + D, cs_k], ka_ps[:])

                groups = [blks[i : i + 2] for i in range(0, len(blks), 2)]
                ctxN = []
                for p in range(2):
                    ctxN_p = nrm.tile([128, CHUNK], F32R, tag=f"ctxN{p}")
                    ctxN.append(ctxN_p)
                    ctx_ps = [
                        pp.tile([D + 1, CHUNK], F32, tag="ctx", name=f"ctx{hh}")
                        for hh in range(2)
                    ]
                    for gi, g in enumerate(groups):
                        s_ps = [
                            sp.tile([128, 2 * CHUNK], F32, tag="scores", name=f"sps{hh}")
                            for hh in range(2)
                        ]
                        es = [
                            es_pool.tile([128, 2 * CHUNK], F32R, tag="es", name=f"es{hh}")
                            for hh in range(2)
                        ]
                        # interleave the two heads of the pair so their K=64
                        # matmuls land on disjoint PE row groups and overlap
                        for j, (blk, mode, aux) in enumerate(g):
                            cm, _ = c0_of(blk, mode)
                            for hh in range(2):
                                o = hh * D
                                nc.tensor.matmul(
                                    s_ps[hh][:, j * CHUNK + cm : (j + 1) * CHUNK],
                                    kAT[p][o : o + D, blk * 128 : (blk + 1) * 128],
                                    qT[p][o : o + D, q0 + cm : q0 + CHUNK],
                                    start=True,
                                    stop=True,
                                )
                        all_plain = all(
                            c0_of(blk, mode)[1] == 0 for blk, mode, _ in g
                        )
                        for hh in range(2):
                            if all_plain:
                                w = len(g) * CHUNK
                                nc.scalar.activation(
                                    es[hh][:, :w], s_ps[hh][:, :w], Exp
                                )
                            else:
                                for j, (blk, mode, aux) in enumerate(g):
                                    cm, c0 = c0_of(blk, mode)
                                    js0 = j * CHUNK
                                    nc.scalar.activation(
                                        es[hh][:, js0 + c0 : js0 + CHUNK],
                                        s_ps[hh][:, js0 + c0 : js0 + CHUNK],
                                        Exp,
                                    )
                            for j, (blk, mode, aux) in enumerate(g):
                                cm, c0 = c0_of(blk, mode)
                                js0 = j * CHUNK
                                if mode == "causal":
                                    # one select covers the dead prefix (fill
                                    # writes 0 over uninit data) + triangle
                                    nc.gpsimd.affine_select(
                                        es[hh][:, js0 : js0 + c0 + 128],
                                        es[hh][:, js0 : js0 + c0 + 128],
                                        pattern=[[1, c0 + 128]],
                                        compare_op=mybir.AluOpType.is_ge,
                                        fill=0.0,
                                        base=q0 - blk * 128,
                                        channel_multiplier=-1,
                                    )
                                elif mode == "tile":
                                    nc.vector.tensor_mul(
                                        es[hh][:, js0 : js0 + CHUNK],
                                        es[hh][:, js0 : js0 + CHUNK],
                                        mtiles[aux][:],
                                    )
                        for hh in range(2):
                            h = 2 * p + hh
                            for j, (blk, mode, aux) in enumerate(g):
                                cm, _ = c0_of(blk, mode)
                                nc.tensor.matmul(
                                    ctx_ps[hh][:, cm:],
                                    va[h][:, blk * (D + 1) : (blk + 1) * (D + 1)],
                                    es[hh][:, j * CHUNK + cm : j * CHUNK + CHUNK],
                                    start=(gi == 0 and j == 0),
                                    stop=(
                                        gi == len(groups) - 1 and j == len(g) - 1
                                    ),
                                )
                    # normalize: ctxN = ctxU * (1/r) broadcast
                    for hh in range(2):
                        o = hh * D
                        r_inv = nrm.tile([1, CHUNK], F32, tag="rinv")
                        nc.vector.reciprocal(r_inv[:], ctx_ps[hh][D : D + 1, :])
                        r_bc = nrm.tile([D, CHUNK], F32, tag="rbc")
                        nc.gpsimd.partition_broadcast(r_bc[:], r_inv[:])
                        nc.vector.tensor_mul(
                            ctxN_p[o : o + D, :], ctx_ps[hh][0:D, :], r_bc[:]
                        )
                    if OUTP_DELAY and p == 0 and prev_out is not None:
                        # overlap previous chunk's output projection with this
                        # chunk's second pair
                        emit_outp(*prev_out)
                        prev_out = None
                if OUTP_DELAY:
                    prev_out = (ctxN, q0)
                else:
                    emit_outp(ctxN, q0)
            if prev_out is not None:
                emit_outp(*prev_out)

        if hw_loop:
            _emit_prep()
            with tc.For_i(0, hw_loop) as _i:
                _emit_body(False)
        else:
            for _rep in range(repeat):
                _emit_body(_rep == 0)

    nc.compile()
    return nc


def prepare(key, query, value, mask, Wq, Wk, Wv, Wo, bo, build=True):
    """Host-side sharding/layout prep. Returns (nc, in_maps, gather)."""
    key = np.asarray(key, dtype=np.float32)
    query = np.asarray(query, dtype=np.float32)
    value = np.asarray(value, dtype=np.float32)
    Wq = np.asarray(Wq, dtype=np.float32)
    Wk = np.asarray(Wk, dtype=np.float32)
    Wv = np.asarray(Wv, dtype=np.float32)
    Wo = np.asarray(Wo, dtype=np.float32)
    bo = np.asarray(bo, dtype=np.float32)

    schedule, mtiles = _analyze_mask(mask)
    nc = build_nc(schedule, len(mtiles)) if build else None

    woT_all = np.ascontiguousarray(Wo.T.reshape(H, D, E))  # per head: Wo[:, cols_h].T
    bo4 = (bo / 4.0).reshape(1, E)
    mt = (
        np.stack(mtiles).astype(np.float32)
        if mtiles
        else None
    )

    in_maps = []
    for c in range(N_CORES):
        b = c // 4
        h0 = 4 * (c % 4)
        hs = slice(h0, h0 + 4)
        q = query[b].reshape(S, H, D)[:, hs, :]  # [S, 4, D]
        k = key[b].reshape(S, H, D)[:, hs, :]
        v = value[b].reshape(S, H, D)[:, hs, :]
        # pair-stacked transposed layouts [2, 128, S]
        qT = np.ascontiguousarray(
            q.transpose(1, 2, 0).reshape(2, 2 * D, S)
        )
        kT = np.ascontiguousarray(k.transpose(1, 2, 0).reshape(2, 2 * D, S))
        va = np.ones((4, S, D + 1), dtype=np.float32)
        va[:, :, :D] = v.transpose(1, 0, 2)
        # partition-major: [4, S, D+1] -> [4, 128, N_BLK*(D+1)]
        va = va.reshape(4, N_BLK, 128, D + 1).transpose(0, 2, 1, 3).reshape(
            4, 128, N_BLK * (D + 1)
        )
        m = {
            "qT": qT,
            "kT": kT,
            "va": np.ascontiguousarray(va),
            "wq": Wq,
            "wk": Wk,
            "wv": Wv,
            "woT": woT_all[h0 : h0 + 4],
            "bo4": bo4,
        }
        if mt is not None:
            m["mtiles"] = mt
        in_maps.append(m)

    def gather(results):
        out = np.empty((B, S, E), dtype=np.float32)
        for b in range(B):
            acc = results[4 * b]["out"].astype(np.float32).copy()
            for c in range(4 * b + 1, 4 * b + 4):
                acc += results[c]["out"]
            out[b] = acc
        return out

    return nc, in_maps, gather


def kernel(key, query, value, mask, Wq, Wk, Wv, Wo, bo):
    nc, in_maps, gather = prepare(key, query, value, mask, Wq, Wk, Wv, Wo, bo)
    res = run_bass_kernel_spmd(nc, in_maps, core_ids=list(range(N_CORES)))
    return gather(res.results)



# revision 12
# speedup vs baseline: 1.3106x; 1.3106x over previous
"""MultiHeadAttention Trainium2 kernel.

B=2, S=2048, E=1024, H=16, D=64. 8 NeuronCores.

Sharding: B*H = 32 (batch, head) pairs -> 4 heads per core (core c handles
batch c//4, heads 4*(c%4)..4*(c%4)+3). Out-projection is column-sharded by
head (Wo folded with Wv); partial [S, E] outputs are summed on host (the
"all-reduce"), each core adding bo/4 so the sum carries the bias exactly once.

Math (per head h):
  S_scores = (q @ Wq.T) @ (k @ Wk.T).T / sqrt(D)  ==  q @ (A/8) @ k.T,
    A = Wq.T @ Wk  (so q needs no projection on device)
  P = softmax(mask(S_scores))  (unnormalized exp + ones-column trick)
  ctx = P @ v  (raw v; Wv folded into Wo)
  out_h = ctx @ (Wo[:, cols_h] @ Wv).T

Device layout: scores computed transposed, S.T[sk, sq] tiles, so that
exp(S.T) feeds the ctx matmul directly as the moving operand and the
ones-column of v_aug produces the softmax denominators r[sq] as row 64 of
the ctx accumulator.

Schedule: the whole core's work is a flat sequence of "units", one per
(chunk, head-pair, sk-block). A unit's score matmuls for BOTH heads of the
pair land side by side in one [128, 1024] PSUM tile so exp is a single
activation instruction. The emission pipeline runs the PE one unit ahead
of the ctx matmuls (scores(u+1) before ctx(u)) so the tensor engine never
drains and can hold its high p-state; out-projection and next-chunk k@A
matmuls are spread between units. Causal masking is a DVE multiply with
two canonical 0/1 tiles (after exp); softmax reciprocal uses the
single-instruction approx DVE op.
"""

import sys

if "/opt/trn_rl_repo" not in sys.path:
    sys.path.insert(0, "/opt/trn_rl_repo")

from collections import deque

import numpy as np

import concourse.bass as bass
import concourse.tile as tile
from concourse import bacc, mybir
from concourse.bass_utils import run_bass_kernel_spmd

B, S, E, H = 2, 2048, 1024, 16
D = E // H  # 64
N_CORES = 8
HEADS_PER_CORE = H * B // N_CORES  # 4
N_CHUNK = 4  # sq chunks of 512
CHUNK = S // N_CHUNK  # 512
N_BLK = S // 128  # 16 sk blocks of 128
F32 = mybir.dt.float32
F32R = mybir.dt.float32r


def _analyze_mask(mask):
    """Classify each (sq-chunk, sk-block) region of the shared mask.

    Returns (schedule, tiles): schedule[ci] is a list of (blk, mode, aux)
    with mode in {"plain", "causal", "tile"}; tiles is the list of distinct
    float32 [128, CHUNK] (sk, sq) multiplicative mask tiles for "tile" mode.
    """
    m = np.asarray(mask).reshape(S, S) != 0
    schedule = []
    tiles = []
    tile_index = {}
    for ci in range(N_CHUNK):
        q0 = ci * CHUNK
        blks = []
        for k in range(N_BLK):
            k0 = k * 128
            mb = m[q0 : q0 + CHUNK, k0 : k0 + 128]  # [sq, sk]
            if not mb.any():
                continue
            if mb.all():
                blks.append((k, "plain", None))
                continue
            causal = (
                np.arange(q0, q0 + CHUNK)[:, None] >= np.arange(k0, k0 + 128)[None, :]
            )
            if np.array_equal(mb, causal):
                blks.append((k, "causal", None))
            else:
                t = np.ascontiguousarray(mb.T.astype(np.float32))  # [sk, sq]
                key = t.tobytes()
                if key not in tile_index:
                    tile_index[key] = len(tiles)
                    tiles.append(t)
                blks.append((k, "tile", tile_index[key]))
        schedule.append(blks)
    return schedule, tiles


def build_nc(schedule, n_mask_tiles, repeat=1, hw_loop=0):
    """Build the SPMD Bass program (identical for all 8 cores).

    repeat>1 / hw_loop>0 re-execute the whole data path (input DMAs
    included) that many times in one NEFF; used by test.py to measure
    per-execution device time as a wall-clock slope.
    """
    nc = bacc.Bacc(
        "TRN2", target_bir_lowering=False, debug=False, num_devices=N_CORES
    )

    qT_d = nc.dram_tensor("qT", [2, 128, S], F32, kind="ExternalInput").ap()
    kT_d = nc.dram_tensor("kT", [2, 128, S], F32, kind="ExternalInput").ap()
    va_d = nc.dram_tensor("va", [4, 128, N_BLK * (D + 1)], F32, kind="ExternalInput").ap()
    wq_d = nc.dram_tensor("wq", [D, D], F32, kind="ExternalInput").ap()
    wk_d = nc.dram_tensor("wk", [D, D], F32, kind="ExternalInput").ap()
    wv_d = nc.dram_tensor("wv", [D, D], F32, kind="ExternalInput").ap()
    woT_d = nc.dram_tensor("woT", [4, D, E], F32, kind="ExternalInput").ap()
    bo4_d = nc.dram_tensor("bo4", [1, E], F32, kind="ExternalInput").ap()
    cm_d = nc.dram_tensor("cmask", [128, 768], F32, kind="ExternalInput").ap()
    if n_mask_tiles:
        mt_d = nc.dram_tensor(
            "mtiles", [n_mask_tiles, 128, CHUNK], F32, kind="ExternalInput"
        ).ap()
    out_d = nc.dram_tensor("out", [S, E], F32, kind="ExternalOutput").ap()
    import os as _os

    _dbg = bool(int(_os.environ.get("K_DEBUG", "0"))) and not hw_loop and repeat == 1
    if _dbg:
        dbg_kat_d = nc.dram_tensor("dbg_kat", [128, S], F32, kind="ExternalOutput").ap()
        dbg_es_d = nc.dram_tensor("dbg_es", [128, 1024], F32, kind="ExternalOutput").ap()
        dbg_r_d = nc.dram_tensor("dbg_r", [1, CHUNK], F32, kind="ExternalOutput").ap()
        dbg_cn_d = nc.dram_tensor("dbg_cn", [128, CHUNK], F32, kind="ExternalOutput").ap()

    Exp = mybir.ActivationFunctionType.Exp
    MUL = mybir.AluOpType.mult

    from contextlib import ExitStack

    with tile.TileContext(nc) as tc, ExitStack() as ctx:
        const = ctx.enter_context(tc.tile_pool(name="const", bufs=1))
        qk = ctx.enter_context(tc.tile_pool(name="qk", bufs=1))
        va_pool = ctx.enter_context(tc.tile_pool(name="vap", bufs=1))
        es_pool = ctx.enter_context(tc.tile_pool(name="es", bufs=6))
        nrm = ctx.enter_context(tc.tile_pool(name="nrm", bufs=2))
        outp = ctx.enter_context(tc.tile_pool(name="outp", bufs=3))
        # PSUM: sp 2x[128,1024] (4 banks) + cxp h0,h1 (2) + mp o,ka (2) = 8
        sp = ctx.enter_context(tc.tile_pool(name="sp", bufs=2, space="PSUM"))
        cxp = ctx.enter_context(tc.tile_pool(name="cxp", bufs=1, space="PSUM"))
        mp = ctx.enter_context(tc.tile_pool(name="mp", bufs=1, space="PSUM"))

        # ---- constants / weight prep ----
        wq_sb = const.tile([D, D], F32, tag="wq")
        # Wk loaded twice side by side: the A.T matmul then yields A.T
        # replicated on partitions 0-63 and 64-127 in one shot (matches
        # either head of a pair-stacked rhs, no SBUF->SBUF copy needed)
        wk2_sb = const.tile([D, 2 * D], F32, tag="wk2")
        wv_sb = const.tile([D, D], F32R, tag="wv")
        nc.sync.dma_start(wq_sb[:], wq_d[:])
        nc.sync.dma_start(wk2_sb[:, 0:D], wk_d[:])
        nc.sync.dma_start(wk2_sb[:, D : 2 * D], wk_d[:])
        nc.sync.dma_start(wv_sb[:], wv_d[:].bitcast(F32R))

        # A.T/8 = (Wk.T @ Wq)/8  [d', d], replicated over both partition halves
        at_ps = mp.tile([128, D], F32, tag="o")
        nc.tensor.matmul(at_ps[:], wk2_sb[:], wq_sb[:], start=True, stop=True)
        at_sb = const.tile([128, D], F32R, tag="at")
        nc.vector.tensor_scalar_mul(at_sb[:], at_ps[:], 1.0 / np.sqrt(float(D)))

        cmask_sb = const.tile([128, 768], F32R, tag="cmask")
        nc.gpsimd.dma_start(cmask_sb[:], cm_d[:].bitcast(F32R))

        wovT, mtiles = [], []
        bo4_bc = None

        def _emit_prep():
            nonlocal bo4_bc
            # ---- deferred weight prep (not needed until first outP) ----
            for p in range(2):
                wovT_p = const.tile(
                    [128, E], F32R, tag=f"wovT{p}", name=f"wovT{p}"
                )
                wovT.append(wovT_p)
            for h in range(4):
                woT_sb = const.tile([D, E], F32R, tag="woT_ld")
                nc.gpsimd.dma_start(woT_sb[:], woT_d[h].bitcast(F32R))
                p, o = h // 2, (h % 2) * D
                for ec in range(E // 512):
                    wo_ps = mp.tile([D, 512], F32, tag="ka")
                    nc.tensor.matmul(
                        wo_ps[:],
                        wv_sb[:],
                        woT_sb[:, ec * 512 : (ec + 1) * 512],
                        start=True,
                        stop=True,
                    )
                    nc.vector.tensor_copy(
                        wovT[p][o : o + D, ec * 512 : (ec + 1) * 512], wo_ps[:]
                    )
            bo4_sb = const.tile([1, E], F32, tag="bo4")
            nc.gpsimd.dma_start(bo4_sb[:], bo4_d[:])
            bo4_bc = const.tile([128, E], F32, tag="bo4bc")
            nc.gpsimd.partition_broadcast(bo4_bc[:], bo4_sb[:])
            for i in range(n_mask_tiles):
                t = const.tile([128, CHUNK], F32R, tag=f"mt{i}", name=f"mt{i}")
                nc.gpsimd.dma_start(t[:], mt_d[i].bitcast(F32R))
                mtiles.append(t)

        def _emit_body(_first):
            # ---- input loads, ci-major ----
            qT = []
            kAT = []
            va = []
            k_sb_l = []
            for p in range(2):
                qT.append(qk.tile([128, S], F32R, tag=f"qT{p}", name=f"qT{p}"))
                k_sb_l.append(qk.tile([128, S], F32R, tag=f"kT{p}", name=f"kT{p}"))
                kAT.append(qk.tile([128, S], F32R, tag=f"kAT{p}", name=f"kAT{p}"))
            for h in range(4):
                v_sb = va_pool.tile(
                    [128, N_BLK * (D + 1)], F32R, tag=f"va{h}", name=f"va{h}"
                )
                va.append(v_sb)
            for ci in range(N_CHUNK):
                cs = slice(ci * CHUNK, (ci + 1) * CHUNK)
                for p in range(2):
                    nc.sync.dma_start(k_sb_l[p][:, cs], kT_d[p, :, cs].bitcast(F32R))
                    nc.sync.dma_start(qT[p][:, cs], qT_d[p, :, cs].bitcast(F32R))
                if ci < 2:
                    for hh in range(2):
                        h = 2 * ci + hh
                        nc.gpsimd.dma_start(va[h][:], va_d[h].bitcast(F32R))

            if _first and not hw_loop:
                _emit_prep()

            # ---- flat unit list ----
            # unit = (ci, p, blk, mode, aux, first_of_cp, last_of_cp)
            units = []
            for ci in range(N_CHUNK):
                blks = schedule[ci]
                for p in range(2):
                    for bi, (blk, mode, aux) in enumerate(blks):
                        units.append(
                            (ci, p, blk, mode, aux, bi == 0, bi == len(blks) - 1)
                        )

            def c0cm(ci, blk, mode):
                if mode != "causal":
                    return 0, 0
                c0 = max(0, blk * 128 - ci * CHUNK)
                return c0, min(c0, CHUNK - 256)

            # per-unit state handed from scores to ctx
            es_of = {}
            ctx_tiles = {}  # (ci, p) -> [h0_tile, h1_tile]
            ctxN_of = {}  # ci -> [ctxN_p0, ctxN_p1]
            pending = deque()

            def emit_ka(ci):
                cs_k = slice(ci * CHUNK, (ci + 1) * CHUNK)
                for p_ in range(2):
                    for hh in range(2):
                        o = hh * D

                        def ka_thunk(p_=p_, o=o, cs_k=cs_k):
                            ka_ps = mp.tile([D, CHUNK], F32, tag="ka", name="ka_ps")
                            nc.tensor.matmul(
                                ka_ps[:],
                                at_sb[o : o + D, :],
                                k_sb_l[p_][o : o + D, cs_k],
                                start=True,
                                stop=True,
                            )
                            nc.vector.tensor_copy(kAT[p_][o : o + D, cs_k], ka_ps[:])

                        yield ka_thunk

            def emit_scores(u):
                ci, p, blk, mode, aux, first, last = u
                q0 = ci * CHUNK
                c0, cm = c0cm(ci, blk, mode)
                s_ps = sp.tile([128, 2 * CHUNK], F32, tag="s", name="s_ps")
                es = es_pool.tile([128, 2 * CHUNK], F32R, tag="es", name="es")
                es_of[id(u)] = (s_ps, es)
                for hh in range(2):
                    o = hh * D
                    nc.tensor.matmul(
                        s_ps[:, hh * CHUNK + cm : (hh + 1) * CHUNK],
                        kAT[p][o : o + D, blk * 128 : (blk + 1) * 128],
                        qT[p][o : o + D, q0 + cm : q0 + CHUNK],
                        start=True,
                        stop=True,
                    )
                # single exp instruction covering both heads
                _EXP3D = int(_os.environ.get("K_EXP3D", "0"))
                if mode == "causal" and cm > 0:
                    if _EXP3D:
                        es3 = es[:].rearrange("p (h w) -> p h w", h=2)
                        sp3 = s_ps[:].rearrange("p (h w) -> p h w", h=2)
                        nc.scalar.activation(
                            es3[:, :, cm:CHUNK], sp3[:, :, cm:CHUNK], Exp
                        )
                    else:
                        for hh in range(2):
                            js = hh * CHUNK
                            nc.scalar.activation(
                                es[:, js + cm : js + CHUNK],
                                s_ps[:, js + cm : js + CHUNK],
                                Exp,
                            )
                else:
                    nc.scalar.activation(es[:], s_ps[:], Exp)

            def emit_ctx(u):
                ci, p, blk, mode, aux, first, last = u
                c0, cm = c0cm(ci, blk, mode)
                s_ps, es = es_of.pop(id(u))
                _MSK3D = int(_os.environ.get("K_MSK3D", "0"))
                if mode == "causal":
                    # zero the invalid region (post-exp) for both heads at once
                    if _MSK3D:
                        es3 = es[:].rearrange("p (h w) -> p h w", h=2)
                        if c0 < 384:
                            nc.vector.tensor_tensor(
                                es3[:, :, c0 : c0 + 128],
                                es3[:, :, c0 : c0 + 128],
                                cmask_sb[:, 0:256],
                                op=MUL,
                            )
                        else:
                            nc.vector.tensor_tensor(
                                es3[:, :, cm : cm + 256],
                                es3[:, :, cm : cm + 256],
                                cmask_sb[:, 256:768],
                                op=MUL,
                            )
                    else:
                        moff, mw = (0, 128) if c0 < 384 else (256, 256)
                        for hh in range(2):
                            js = hh * CHUNK
                            r0_, r1_ = (c0, c0 + 128) if c0 < 384 else (cm, cm + 256)
                            nc.vector.tensor_tensor(
                                es[:, js + r0_ : js + r1_],
                                es[:, js + r0_ : js + r1_],
                                cmask_sb[:, moff : moff + mw],
                                op=MUL,
                            )
                elif mode == "tile":
                    for hh in range(2):
                        nc.vector.tensor_tensor(
                            es[:, hh * CHUNK : (hh + 1) * CHUNK],
                            es[:, hh * CHUNK : (hh + 1) * CHUNK],
                            mtiles[aux][:],
                            op=MUL,
                        )
                if first:
                    ctx_tiles[(ci, p)] = [
                        cxp.tile([D + 1, CHUNK], F32, tag=f"h{hh}", name=f"ctx{hh}")
                        for hh in range(2)
                    ]
                ctx_ps = ctx_tiles[(ci, p)]
                for hh in range(2):
                    h = 2 * p + hh
                    nc.tensor.matmul(
                        ctx_ps[hh][:, cm:],
                        va[h][:, blk * (D + 1) : (blk + 1) * (D + 1)],
                        es[:, hh * CHUNK + cm : (hh + 1) * CHUNK],
                        start=first,
                        stop=last,
                    )
                if _dbg and ci == 0 and p == 0 and blk == int(_os.environ.get("K_DBG_BLK", "0")):
                    nc.sync.dma_start(dbg_es_d[:], es[:].bitcast(F32))
                if last:
                    emit_normalize(ci, p)

            def emit_normalize(ci, p):
                ctx_ps = ctx_tiles.pop((ci, p))
                ctxN_p = nrm.tile(
                    [128, CHUNK], F32R, tag=f"ctxN{p}", name=f"ctxN{p}"
                )
                ctxN_of.setdefault(ci, [None, None])[p] = ctxN_p
                # denominators: each head's r row (PSUM partition 64) copied to
                # partition 0 of an SBUF tile, then fast-reciprocal. The custom
                # DVE reciprocal and gpsimd partition_broadcast BOTH silently
                # corrupt data when given non-partition-0 operands on hardware,
                # so every step here is partition-0 aligned.
                for hh in range(2):
                    o = hh * D
                    rr = nrm.tile([1, CHUNK], F32, tag="rr")
                    nc.vector.tensor_copy(rr[:], ctx_ps[hh][D : D + 1, :])
                    r_inv = nrm.tile([1, CHUNK], F32, tag="rinv")
                    nc.vector.reciprocal_approx_fast(out=r_inv[:], in_=rr[:])
                    r_bc = nrm.tile([D, CHUNK], F32, tag="rbc")
                    nc.gpsimd.partition_broadcast(r_bc[:], r_inv[:])
                    nc.vector.tensor_tensor(
                        ctxN_p[o : o + D, :], ctx_ps[hh][0:D, :], r_bc[:], op=MUL
                    )
                    if _dbg and ci == 0 and p == 0 and hh == 0:
                        nc.sync.dma_start(dbg_r_d[:], r_inv[:])
                if _dbg and ci == 0 and p == 0:
                    nc.sync.dma_start(dbg_cn_d[:], ctxN_p[:].bitcast(F32))
                    nc.sync.dma_start(dbg_kat_d[:], kAT[0][:].bitcast(F32))
                if p == 1:
                    for i_pc, pc in enumerate(outp_pieces(ci)):
                        pending.append(pc)

            def outp_pieces(ci):
                q0 = ci * CHUNK
                for sb in range(CHUNK // 128):
                    for ec in range(E // 512):

                        def piece(sb=sb, ec=ec, q0=q0, ci=ci):
                            ctxN = ctxN_of[ci]
                            ls = slice(sb * 128, (sb + 1) * 128)
                            es_ = slice(ec * 512, (ec + 1) * 512)
                            tg = "o" if (sb * 2 + ec) % 2 == 0 else "ka"
                            o_ps = mp.tile([128, 512], F32, tag=tg, name="o_ps")
                            nc.tensor.matmul(
                                o_ps[:],
                                ctxN[0][:, ls],
                                wovT[0][:, es_],
                                start=True,
                                stop=False,
                            )
                            nc.tensor.matmul(
                                o_ps[:],
                                ctxN[1][:, ls],
                                wovT[1][:, es_],
                                start=False,
                                stop=True,
                            )
                            o_sb = outp.tile([128, 512], F32, tag="osb", name="o_sb")
                            nc.vector.tensor_tensor(
                                o_sb[:], o_ps[:], bo4_bc[:, es_], op=mybir.AluOpType.add
                            )
                            nc.sync.dma_start(
                                out_d[q0 + sb * 128 : q0 + (sb + 1) * 128, es_],
                                o_sb[:],
                            )

                        yield piece

            # ---- pipelined emission: PE runs one unit ahead of ctx ----
            for t in emit_ka(0):
                t()
            prev = None
            for i, u in enumerate(units):
                ci, p, blk = u[0], u[1], u[2]
                if u[5] and p == 0 and ci + 1 < N_CHUNK and blk == schedule[ci][0][0]:
                    # entering chunk ci: queue kA for chunk ci+1
                    for t in emit_ka(ci + 1):
                        pending.append(t)
                emit_scores(u)
                for _ in range(2):
                    if pending:
                        pending.popleft()()
                if prev is not None:
                    emit_ctx(prev)
                prev = u
            emit_ctx(prev)
            while pending:
                pending.popleft()()

        if hw_loop:
            _emit_prep()
            with tc.For_i(0, hw_loop) as _i:
                _emit_body(False)
        else:
            for _rep in range(repeat):
                _emit_body(_rep == 0)

    nc.compile()
    return nc


def _canonical_cmask():
    i = np.arange(128)[:, None]
    m128 = (np.arange(128)[None, :] >= i).astype(np.float32)
    m256 = (np.arange(256)[None, :] >= i + 128).astype(np.float32)
    return np.concatenate(
        [np.tile(m128, (1, 2)), np.tile(m256, (1, 2))], axis=1
    )  # [128, 768]


def prepare(key, query, value, mask, Wq, Wk, Wv, Wo, bo, build=True):
    """Host-side sharding/layout prep. Returns (nc, in_maps, gather)."""
    key = np.asarray(key, dtype=np.float32)
    query = np.asarray(query, dtype=np.float32)
    value = np.asarray(value, dtype=np.float32)
    Wq = np.asarray(Wq, dtype=np.float32)
    Wk = np.asarray(Wk, dtype=np.float32)
    Wv = np.asarray(Wv, dtype=np.float32)
    Wo = np.asarray(Wo, dtype=np.float32)
    bo = np.asarray(bo, dtype=np.float32)

    schedule, mtiles = _analyze_mask(mask)
    nc = build_nc(schedule, len(mtiles)) if build else None

    woT_all = np.ascontiguousarray(Wo.T.reshape(H, D, E))  # per head: Wo[:, cols_h].T
    bo4 = (bo / 4.0).reshape(1, E)
    cmask = _canonical_cmask()
    mt = np.stack(mtiles).astype(np.float32) if mtiles else None

    in_maps = []
    for c in range(N_CORES):
        b = c // 4
        h0 = 4 * (c % 4)
        hs = slice(h0, h0 + 4)
        q = query[b].reshape(S, H, D)[:, hs, :]  # [S, 4, D]
        k = key[b].reshape(S, H, D)[:, hs, :]
        v = value[b].reshape(S, H, D)[:, hs, :]
        # pair-stacked transposed layouts [2, 128, S]
        qT = np.ascontiguousarray(q.transpose(1, 2, 0).reshape(2, 2 * D, S))
        kT = np.ascontiguousarray(k.transpose(1, 2, 0).reshape(2, 2 * D, S))
        va = np.ones((4, S, D + 1), dtype=np.float32)
        va[:, :, :D] = v.transpose(1, 0, 2)
        # partition-major: [4, S, D+1] -> [4, 128, N_BLK*(D+1)]
        va = va.reshape(4, N_BLK, 128, D + 1).transpose(0, 2, 1, 3).reshape(
            4, 128, N_BLK * (D + 1)
        )
        m = {
            "qT": qT,
            "kT": kT,
            "va": np.ascontiguousarray(va),
            "wq": Wq,
            "wk": Wk,
            "wv": Wv,
            "woT": woT_all[h0 : h0 + 4],
            "bo4": bo4,
            "cmask": cmask,
        }
        if mt is not None:
            m["mtiles"] = mt
        in_maps.append(m)

    def gather(results):
        out = np.empty((B, S, E), dtype=np.float32)
        for b in range(B):
            acc = results[4 * b]["out"].astype(np.float32).copy()
            for c in range(4 * b + 1, 4 * b + 4):
                acc += results[c]["out"]
            out[b] = acc
        return out

    return nc, in_maps, gather


def kernel(key, query, value, mask, Wq, Wk, Wv, Wo, bo):
    nc, in_maps, gather = prepare(key, query, value, mask, Wq, Wk, Wv, Wo, bo)
    res = run_bass_kernel_spmd(nc, in_maps, core_ids=list(range(N_CORES)))
    return gather(res.results)


# revision 13
# speedup vs baseline: 1.3427x; 1.0245x over previous
"""MultiHeadAttention Trainium2 kernel.

B=2, S=2048, E=1024, H=16, D=64. 8 NeuronCores.

Sharding: B*H = 32 (batch, head) pairs -> 4 heads per core (core c handles
batch c//4, heads 4*(c%4)..4*(c%4)+3). Out-projection is column-sharded by
head (Wo folded with Wv); partial [S, E] outputs are summed on host (the
"all-reduce"), each core adding bo/4 so the sum carries the bias exactly once.

Math (per head h):
  S_scores = (q @ Wq.T) @ (k @ Wk.T).T / sqrt(D)  ==  q @ (A/8) @ k.T,
    A = Wq.T @ Wk  (so q needs no projection on device)
  P = softmax(mask(S_scores))  (unnormalized exp + ones-column trick)
  ctx = P @ v  (raw v; Wv folded into Wo)
  out_h = ctx @ (Wo[:, cols_h] @ Wv).T

Device layout: scores computed transposed, S.T[sk, sq] tiles, so that
exp(S.T) feeds the ctx matmul directly as the moving operand and the
ones-column of v_aug produces the softmax denominators r[sq] as row 64 of
the ctx accumulator.

Schedule: the whole core's work is a flat sequence of "units", one per
(chunk, head-pair, sk-block). A unit's score matmuls for BOTH heads of the
pair land side by side in one [128, 1024] PSUM tile so exp is a single
activation instruction. The emission pipeline runs the PE one unit ahead
of the ctx matmuls (scores(u+1) before ctx(u)) so the tensor engine never
drains and can hold its high p-state; out-projection and next-chunk k@A
matmuls are spread between units. Causal masking is a DVE multiply with
two canonical 0/1 tiles (after exp); softmax reciprocal uses the
single-instruction approx DVE op.
"""

import sys

if "/opt/trn_rl_repo" not in sys.path:
    sys.path.insert(0, "/opt/trn_rl_repo")

from collections import deque

import numpy as np

import concourse.bass as bass
import concourse.tile as tile
from concourse import bacc, mybir
from concourse.bass_utils import run_bass_kernel_spmd

B, S, E, H = 2, 2048, 1024, 16
D = E // H  # 64
N_CORES = 8
HEADS_PER_CORE = H * B // N_CORES  # 4
N_CHUNK = 4  # sq chunks of 512
CHUNK = S // N_CHUNK  # 512
N_BLK = S // 128  # 16 sk blocks of 128
F32 = mybir.dt.float32
F32R = mybir.dt.float32r


def _analyze_mask(mask):
    """Classify each (sq-chunk, sk-block) region of the shared mask.

    Returns (schedule, tiles): schedule[ci] is a list of (blk, mode, aux)
    with mode in {"plain", "causal", "tile"}; tiles is the list of distinct
    float32 [128, CHUNK] (sk, sq) multiplicative mask tiles for "tile" mode.
    """
    m = np.asarray(mask).reshape(S, S) != 0
    schedule = []
    tiles = []
    tile_index = {}
    for ci in range(N_CHUNK):
        q0 = ci * CHUNK
        blks = []
        for k in range(N_BLK):
            k0 = k * 128
            mb = m[q0 : q0 + CHUNK, k0 : k0 + 128]  # [sq, sk]
            if not mb.any():
                continue
            if mb.all():
                blks.append((k, "plain", None))
                continue
            causal = (
                np.arange(q0, q0 + CHUNK)[:, None] >= np.arange(k0, k0 + 128)[None, :]
            )
            if np.array_equal(mb, causal):
                blks.append((k, "causal", None))
            else:
                t = np.ascontiguousarray(mb.T.astype(np.float32))  # [sk, sq]
                key = t.tobytes()
                if key not in tile_index:
                    tile_index[key] = len(tiles)
                    tiles.append(t)
                blks.append((k, "tile", tile_index[key]))
        schedule.append(blks)
    return schedule, tiles


def build_nc(schedule, n_mask_tiles, repeat=1, hw_loop=0):
    """Build the SPMD Bass program (identical for all 8 cores).

    repeat>1 / hw_loop>0 re-execute the whole data path (input DMAs
    included) that many times in one NEFF; used by test.py to measure
    per-execution device time as a wall-clock slope.
    """
    nc = bacc.Bacc(
        "TRN2", target_bir_lowering=False, debug=False, num_devices=N_CORES
    )

    qT_d = nc.dram_tensor("qT", [2, 128, S], F32, kind="ExternalInput").ap()
    kT_d = nc.dram_tensor("kT", [2, 128, S], F32, kind="ExternalInput").ap()
    va_d = nc.dram_tensor("va", [4, 128, N_BLK * (D + 1)], F32, kind="ExternalInput").ap()
    wq_d = nc.dram_tensor("wq", [D, D], F32, kind="ExternalInput").ap()
    wk_d = nc.dram_tensor("wk", [D, D], F32, kind="ExternalInput").ap()
    wv_d = nc.dram_tensor("wv", [D, D], F32, kind="ExternalInput").ap()
    woT_d = nc.dram_tensor("woT", [4, D, E], F32, kind="ExternalInput").ap()
    bo4_d = nc.dram_tensor("bo4", [1, E], F32, kind="ExternalInput").ap()
    cm_d = nc.dram_tensor("cmask", [128, 768], F32, kind="ExternalInput").ap()
    if n_mask_tiles:
        mt_d = nc.dram_tensor(
            "mtiles", [n_mask_tiles, 128, CHUNK], F32, kind="ExternalInput"
        ).ap()
    out_d = nc.dram_tensor("out", [S, E], F32, kind="ExternalOutput").ap()
    import os as _os

    _dbg = bool(int(_os.environ.get("K_DEBUG", "0"))) and not hw_loop and repeat == 1
    if _dbg:
        dbg_kat_d = nc.dram_tensor("dbg_kat", [128, S], F32, kind="ExternalOutput").ap()
        dbg_es_d = nc.dram_tensor("dbg_es", [128, 1024], F32, kind="ExternalOutput").ap()
        dbg_r_d = nc.dram_tensor("dbg_r", [1, CHUNK], F32, kind="ExternalOutput").ap()
        dbg_cn_d = nc.dram_tensor("dbg_cn", [128, CHUNK], F32, kind="ExternalOutput").ap()

    Exp = mybir.ActivationFunctionType.Exp
    MUL = mybir.AluOpType.mult

    from contextlib import ExitStack

    with tile.TileContext(nc) as tc, ExitStack() as ctx:
        const = ctx.enter_context(tc.tile_pool(name="const", bufs=1))
        qk = ctx.enter_context(tc.tile_pool(name="qk", bufs=1))
        va_pool = ctx.enter_context(tc.tile_pool(name="vap", bufs=1))
        es_pool = ctx.enter_context(tc.tile_pool(name="es", bufs=6))
        nrm = ctx.enter_context(tc.tile_pool(name="nrm", bufs=2))
        outp = ctx.enter_context(tc.tile_pool(name="outp", bufs=3))
        # PSUM: sp 2x[128,1024] (4 banks) + cxp h0,h1 (2) + mp o,ka (2) = 8
        sp = ctx.enter_context(tc.tile_pool(name="sp", bufs=2, space="PSUM"))
        cxp = ctx.enter_context(tc.tile_pool(name="cxp", bufs=1, space="PSUM"))
        mp = ctx.enter_context(tc.tile_pool(name="mp", bufs=1, space="PSUM"))

        # ---- constants / weight prep ----
        wq_sb = const.tile([D, D], F32, tag="wq")
        # Wk loaded twice side by side: the A.T matmul then yields A.T
        # replicated on partitions 0-63 and 64-127 in one shot (matches
        # either head of a pair-stacked rhs, no SBUF->SBUF copy needed)
        wk2_sb = const.tile([D, 2 * D], F32, tag="wk2")
        wv_sb = const.tile([D, D], F32R, tag="wv")
        nc.sync.dma_start(wq_sb[:], wq_d[:])
        nc.sync.dma_start(wk2_sb[:, 0:D], wk_d[:])
        nc.sync.dma_start(wk2_sb[:, D : 2 * D], wk_d[:])
        nc.sync.dma_start(wv_sb[:], wv_d[:].bitcast(F32R))

        # A.T/8 = (Wk.T @ Wq)/8  [d', d], replicated over both partition halves
        at_ps = mp.tile([128, D], F32, tag="o")
        nc.tensor.matmul(at_ps[:], wk2_sb[:], wq_sb[:], start=True, stop=True)
        at_sb = const.tile([128, D], F32R, tag="at")
        nc.vector.tensor_scalar_mul(at_sb[:], at_ps[:], 1.0 / np.sqrt(float(D)))

        cmask_sb = const.tile([128, 768], F32R, tag="cmask")
        nc.gpsimd.dma_start(cmask_sb[:], cm_d[:].bitcast(F32R))

        wovT, mtiles = [], []
        bo4_bc = None

        def _emit_prep():
            nonlocal bo4_bc
            # ---- deferred weight prep (not needed until first outP) ----
            for p in range(2):
                wovT_p = const.tile(
                    [128, E], F32R, tag=f"wovT{p}", name=f"wovT{p}"
                )
                wovT.append(wovT_p)
            for h in range(4):
                woT_sb = const.tile([D, E], F32R, tag="woT_ld")
                nc.gpsimd.dma_start(woT_sb[:], woT_d[h].bitcast(F32R))
                p, o = h // 2, (h % 2) * D
                for ec in range(E // 512):
                    wo_ps = mp.tile([D, 512], F32, tag="ka")
                    nc.tensor.matmul(
                        wo_ps[:],
                        wv_sb[:],
                        woT_sb[:, ec * 512 : (ec + 1) * 512],
                        start=True,
                        stop=True,
                    )
                    nc.vector.tensor_copy(
                        wovT[p][o : o + D, ec * 512 : (ec + 1) * 512], wo_ps[:]
                    )
            bo4_sb = const.tile([1, E], F32, tag="bo4")
            nc.gpsimd.dma_start(bo4_sb[:], bo4_d[:])
            bo4_bc = const.tile([128, E], F32, tag="bo4bc")
            nc.gpsimd.partition_broadcast(bo4_bc[:], bo4_sb[:])
            for i in range(n_mask_tiles):
                t = const.tile([128, CHUNK], F32R, tag=f"mt{i}", name=f"mt{i}")
                nc.gpsimd.dma_start(t[:], mt_d[i].bitcast(F32R))
                mtiles.append(t)

        def _emit_body(_first):
            # ---- input loads, ci-major ----
            qT = []
            kAT = []
            va = []
            k_sb_l = []
            for p in range(2):
                qT.append(qk.tile([128, S], F32R, tag=f"qT{p}", name=f"qT{p}"))
                k_sb_l.append(qk.tile([128, S], F32R, tag=f"kT{p}", name=f"kT{p}"))
                kAT.append(qk.tile([128, S], F32R, tag=f"kAT{p}", name=f"kAT{p}"))
            for h in range(4):
                v_sb = va_pool.tile(
                    [128, N_BLK * (D + 1)], F32R, tag=f"va{h}", name=f"va{h}"
                )
                va.append(v_sb)
            for ci in range(N_CHUNK):
                cs = slice(ci * CHUNK, (ci + 1) * CHUNK)
                for p in range(2):
                    nc.sync.dma_start(k_sb_l[p][:, cs], kT_d[p, :, cs].bitcast(F32R))
                    nc.sync.dma_start(qT[p][:, cs], qT_d[p, :, cs].bitcast(F32R))
                if ci < 2:
                    for hh in range(2):
                        h = 2 * ci + hh
                        nc.gpsimd.dma_start(va[h][:], va_d[h].bitcast(F32R))

            if _first and not hw_loop:
                _emit_prep()

            # ---- flat unit list ----
            # unit = (ci, p, blk, mode, aux, first_of_cp, last_of_cp)
            units = []
            for ci in range(N_CHUNK):
                blks = schedule[ci]
                for p in range(2):
                    for bi, (blk, mode, aux) in enumerate(blks):
                        units.append(
                            (ci, p, blk, mode, aux, bi == 0, bi == len(blks) - 1)
                        )

            def c0cm(ci, blk, mode):
                if mode != "causal":
                    return 0, 0
                c0 = max(0, blk * 128 - ci * CHUNK)
                return c0, min(c0, CHUNK - 256)

            # per-unit state handed from scores to ctx
            es_of = {}
            ctx_tiles = {}  # (ci, p) -> [h0_tile, h1_tile]
            ctxN_of = {}  # ci -> [ctxN_p0, ctxN_p1]
            pending = deque()

            def emit_ka(ci):
                cs_k = slice(ci * CHUNK, (ci + 1) * CHUNK)
                for p_ in range(2):
                    for hh in range(2):
                        o = hh * D

                        def ka_thunk(p_=p_, o=o, cs_k=cs_k):
                            ka_ps = mp.tile([D, CHUNK], F32, tag="ka", name="ka_ps")
                            nc.tensor.matmul(
                                ka_ps[:],
                                at_sb[o : o + D, :],
                                k_sb_l[p_][o : o + D, cs_k],
                                start=True,
                                stop=True,
                            )
                            nc.vector.tensor_copy(kAT[p_][o : o + D, cs_k], ka_ps[:])

                        yield ka_thunk

            def emit_scores(u):
                ci, p, blk, mode, aux, first, last = u
                q0 = ci * CHUNK
                c0, cm = c0cm(ci, blk, mode)
                s_ps = sp.tile([128, 2 * CHUNK], F32, tag="s", name="s_ps")
                es = es_pool.tile([128, 2 * CHUNK], F32R, tag="es", name="es")
                es_of[id(u)] = (s_ps, es)
                for hh in range(2):
                    o = hh * D
                    nc.tensor.matmul(
                        s_ps[:, hh * CHUNK + cm : (hh + 1) * CHUNK],
                        kAT[p][o : o + D, blk * 128 : (blk + 1) * 128],
                        qT[p][o : o + D, q0 + cm : q0 + CHUNK],
                        start=True,
                        stop=True,
                    )
                # single exp instruction covering both heads
                _EXP3D = int(_os.environ.get("K_EXP3D", "1"))
                if mode == "causal" and cm > 0:
                    if _EXP3D:
                        es3 = es[:].rearrange("p (h w) -> p h w", h=2)
                        sp3 = s_ps[:].rearrange("p (h w) -> p h w", h=2)
                        nc.scalar.activation(
                            es3[:, :, cm:CHUNK], sp3[:, :, cm:CHUNK], Exp
                        )
                    else:
                        for hh in range(2):
                            js = hh * CHUNK
                            nc.scalar.activation(
                                es[:, js + cm : js + CHUNK],
                                s_ps[:, js + cm : js + CHUNK],
                                Exp,
                            )
                else:
                    nc.scalar.activation(es[:], s_ps[:], Exp)

            def emit_ctx(u):
                ci, p, blk, mode, aux, first, last = u
                c0, cm = c0cm(ci, blk, mode)
                s_ps, es = es_of.pop(id(u))
                _MSK3D = int(_os.environ.get("K_MSK3D", "1"))
                if mode == "causal":
                    # zero the invalid region (post-exp) for both heads at once
                    if _MSK3D:
                        es3 = es[:].rearrange("p (h w) -> p h w", h=2)
                        if c0 < 384:
                            nc.vector.tensor_tensor(
                                es3[:, :, c0 : c0 + 128],
                                es3[:, :, c0 : c0 + 128],
                                cmask_sb[:, 0:256],
                                op=MUL,
                            )
                        else:
                            nc.vector.tensor_tensor(
                                es3[:, :, cm : cm + 256],
                                es3[:, :, cm : cm + 256],
                                cmask_sb[:, 256:768],
                                op=MUL,
                            )
                    else:
                        moff, mw = (0, 128) if c0 < 384 else (256, 256)
                        for hh in range(2):
                            js = hh * CHUNK
                            r0_, r1_ = (c0, c0 + 128) if c0 < 384 else (cm, cm + 256)
                            nc.vector.tensor_tensor(
                                es[:, js + r0_ : js + r1_],
                                es[:, js + r0_ : js + r1_],
                                cmask_sb[:, moff : moff + mw],
                                op=MUL,
                            )
                elif mode == "tile":
                    for hh in range(2):
                        nc.vector.tensor_tensor(
                            es[:, hh * CHUNK : (hh + 1) * CHUNK],
                            es[:, hh * CHUNK : (hh + 1) * CHUNK],
                            mtiles[aux][:],
                            op=MUL,
                        )
                if first:
                    ctx_tiles[(ci, p)] = [
                        cxp.tile([D + 1, CHUNK], F32, tag=f"h{hh}", name=f"ctx{hh}")
                        for hh in range(2)
                    ]
                ctx_ps = ctx_tiles[(ci, p)]
                for hh in range(2):
                    h = 2 * p + hh
                    nc.tensor.matmul(
                        ctx_ps[hh][:, cm:],
                        va[h][:, blk * (D + 1) : (blk + 1) * (D + 1)],
                        es[:, hh * CHUNK + cm : (hh + 1) * CHUNK],
                        start=first,
                        stop=last,
                    )
                if _dbg and ci == 0 and p == 0 and blk == int(_os.environ.get("K_DBG_BLK", "0")):
                    nc.sync.dma_start(dbg_es_d[:], es[:].bitcast(F32))
                if last:
                    emit_normalize(ci, p)

            def emit_normalize(ci, p):
                ctx_ps = ctx_tiles.pop((ci, p))
                ctxN_p = nrm.tile(
                    [128, CHUNK], F32R, tag=f"ctxN{p}", name=f"ctxN{p}"
                )
                ctxN_of.setdefault(ci, [None, None])[p] = ctxN_p
                # denominators: each head's r row (PSUM partition 64) copied to
                # partition 0 of an SBUF tile, then fast-reciprocal. The custom
                # DVE reciprocal and gpsimd partition_broadcast BOTH silently
                # corrupt data when given non-partition-0 operands on hardware,
                # so every step here is partition-0 aligned.
                for hh in range(2):
                    o = hh * D
                    rr = nrm.tile([1, CHUNK], F32, tag="rr")
                    nc.vector.tensor_copy(rr[:], ctx_ps[hh][D : D + 1, :])
                    r_inv = nrm.tile([1, CHUNK], F32, tag="rinv")
                    nc.vector.reciprocal_approx_fast(out=r_inv[:], in_=rr[:])
                    r_bc = nrm.tile([D, CHUNK], F32, tag="rbc")
                    nc.gpsimd.partition_broadcast(r_bc[:], r_inv[:])
                    nc.vector.tensor_tensor(
                        ctxN_p[o : o + D, :], ctx_ps[hh][0:D, :], r_bc[:], op=MUL
                    )
                    if _dbg and ci == 0 and p == 0 and hh == 0:
                        nc.sync.dma_start(dbg_r_d[:], r_inv[:])
                if _dbg and ci == 0 and p == 0:
                    nc.sync.dma_start(dbg_cn_d[:], ctxN_p[:].bitcast(F32))
                    nc.sync.dma_start(dbg_kat_d[:], kAT[0][:].bitcast(F32))
                if p == 1:
                    for i_pc, pc in enumerate(outp_pieces(ci)):
                        pending.append(pc)

            def outp_pieces(ci):
                q0 = ci * CHUNK
                for sb in range(CHUNK // 128):
                    for ec in range(E // 512):

                        def piece(sb=sb, ec=ec, q0=q0, ci=ci):
                            ctxN = ctxN_of[ci]
                            ls = slice(sb * 128, (sb + 1) * 128)
                            es_ = slice(ec * 512, (ec + 1) * 512)
                            tg = "o" if (sb * 2 + ec) % 2 == 0 else "ka"
                            o_ps = mp.tile([128, 512], F32, tag=tg, name="o_ps")
                            nc.tensor.matmul(
                                o_ps[:],
                                ctxN[0][:, ls],
                                wovT[0][:, es_],
                                start=True,
                                stop=False,
                            )
                            nc.tensor.matmul(
                                o_ps[:],
                                ctxN[1][:, ls],
                                wovT[1][:, es_],
                                start=False,
                                stop=True,
                            )
                            o_sb = outp.tile([128, 512], F32, tag="osb", name="o_sb")
                            nc.vector.tensor_tensor(
                                o_sb[:], o_ps[:], bo4_bc[:, es_], op=mybir.AluOpType.add
                            )
                            nc.sync.dma_start(
                                out_d[q0 + sb * 128 : q0 + (sb + 1) * 128, es_],
                                o_sb[:],
                            )

                        yield piece

            # ---- pipelined emission: PE runs one unit ahead of ctx ----
            for t in emit_ka(0):
                t()
            prev = None
            for i, u in enumerate(units):
                ci, p, blk = u[0], u[1], u[2]
                if u[5] and p == 0 and ci + 1 < N_CHUNK and blk == schedule[ci][0][0]:
                    # entering chunk ci: queue kA for chunk ci+1
                    for t in emit_ka(ci + 1):
                        pending.append(t)
                emit_scores(u)
                for _ in range(2):
                    if pending:
                        pending.popleft()()
                if prev is not None:
                    emit_ctx(prev)
                prev = u
            emit_ctx(prev)
            while pending:
                pending.popleft()()

        if hw_loop:
            _emit_prep()
            with tc.For_i(0, hw_loop) as _i:
                _emit_body(False)
        else:
            for _rep in range(repeat):
                _emit_body(_rep == 0)

    nc.compile()
    return nc


def _canonical_cmask():
    i = np.arange(128)[:, None]
    m128 = (np.arange(128)[None, :] >= i).astype(np.float32)
    m256 = (np.arange(256)[None, :] >= i + 128).astype(np.float32)
    return np.concatenate(
        [np.tile(m128, (1, 2)), np.tile(m256, (1, 2))], axis=1
    )  # [128, 768]


def prepare(key, query, value, mask, Wq, Wk, Wv, Wo, bo, build=True):
    """Host-side sharding/layout prep. Returns (nc, in_maps, gather)."""
    key = np.asarray(key, dtype=np.float32)
    query = np.asarray(query, dtype=np.float32)
    value = np.asarray(value, dtype=np.float32)
    Wq = np.asarray(Wq, dtype=np.float32)
    Wk = np.asarray(Wk, dtype=np.float32)
    Wv = np.asarray(Wv, dtype=np.float32)
    Wo = np.asarray(Wo, dtype=np.float32)
    bo = np.asarray(bo, dtype=np.float32)

    schedule, mtiles = _analyze_mask(mask)
    nc = build_nc(schedule, len(mtiles)) if build else None

    woT_all = np.ascontiguousarray(Wo.T.reshape(H, D, E))  # per head: Wo[:, cols_h].T
    bo4 = (bo / 4.0).reshape(1, E)
    cmask = _canonical_cmask()
    mt = np.stack(mtiles).astype(np.float32) if mtiles else None

    in_maps = []
    for c in range(N_CORES):
        b = c // 4
        h0 = 4 * (c % 4)
        hs = slice(h0, h0 + 4)
        q = query[b].reshape(S, H, D)[:, hs, :]  # [S, 4, D]
        k = key[b].reshape(S, H, D)[:, hs, :]
        v = value[b].reshape(S, H, D)[:, hs, :]
        # pair-stacked transposed layouts [2, 128, S]
        qT = np.ascontiguousarray(q.transpose(1, 2, 0).reshape(2, 2 * D, S))
        kT = np.ascontiguousarray(k.transpose(1, 2, 0).reshape(2, 2 * D, S))
        va = np.ones((4, S, D + 1), dtype=np.float32)
        va[:, :, :D] = v.transpose(1, 0, 2)
        # partition-major: [4, S, D+1] -> [4, 128, N_BLK*(D+1)]
        va = va.reshape(4, N_BLK, 128, D + 1).transpose(0, 2, 1, 3).reshape(
            4, 128, N_BLK * (D + 1)
        )
        m = {
            "qT": qT,
            "kT": kT,
            "va": np.ascontiguousarray(va),
            "wq": Wq,
            "wk": Wk,
            "wv": Wv,
            "woT": woT_all[h0 : h0 + 4],
            "bo4": bo4,
            "cmask": cmask,
        }
        if mt is not None:
            m["mtiles"] = mt
        in_maps.append(m)

    def gather(results):
        out = np.empty((B, S, E), dtype=np.float32)
        for b in range(B):
            acc = results[4 * b]["out"].astype(np.float32).copy()
            for c in range(4 * b + 1, 4 * b + 4):
                acc += results[c]["out"]
            out[b] = acc
        return out

    return nc, in_maps, gather


def kernel(key, query, value, mask, Wq, Wk, Wv, Wo, bo):
    nc, in_maps, gather = prepare(key, query, value, mask, Wq, Wk, Wv, Wo, bo)
    res = run_bass_kernel_spmd(nc, in_maps, core_ids=list(range(N_CORES)))
    return gather(res.results)
